# revision 14
# baseline (speedup 1.0000x reference)
"""AdaptiveRankingLoss on 8 Trainium2 NeuronCores (Bass/Tile), upper-triangle v3.

Math
----
reference:  loss = sum_{i<j, |t_i-t_j|>=0.05} 0.5*(w_i+w_j)*relu(-sign(td)*pd + m) / count
            td = t_i - t_j, pd = p_i - p_j, m = ms*0.08*clip(|td|, 0.1, 1.0)

Every per-pair factor is symmetric in i<->j, so each unordered pair is computed
once.  The 64x64 grid of 128-row blocks is covered by a circulant schedule:
row-block I processes column-blocks J in the wrapped window [I, I+n_I) mod 64,
n_I = 33 for I<=31 and 32 for I>=32; every unordered block pair lands in
exactly one window (pair {I,J}, d=J-I: d<=32 -> I's window, else J's), and the
diagonal block leads each window (strict-upper mask there).  Core k owns blocks
{4k..4k+3} and {32+4k..32+4k+3}: identical shapes and work on every core.

Column data is laid out per-core ROTATED by 4k blocks, with the first 3 blocks
duplicated as a tail, so every window is one contiguous slice of a single
[128, 8576] broadcast tile per tensor (slot i<=3: start 128*i len 4224;
slot i>=4: start 4096+128*(i-4) len 4096).

Per block (rows on partitions, window cols on free), all tensors bf16:
    ACT: ad  = Abs( tq_j - tq_i )     tq = bf16(0.08*ms*t)
    ACT: s   = Sign( tq_i - tq_j )
    DVE: m   = (ad max lo) min hi     margin (lo=0.008ms, hi=0.08ms)
    DVE: v   = (ad is_ge theta)      theta = 0.004ms  (<=> |td| >= 0.05)
    DVE: pd  = pq_j - pq_i           [movable to ACT per-block]
    DVE: q   = pd * s
    DVE: vp  = q + m
    ACT: viol= Relu(vp)              [movable to DVE per-block]
    DVE: vm  = v[:,0:128] * U        strict-upper diag mask
    DVE: g   = viol * v              (vm on the leading 128 cols)
    DVE: gw  = g * wc_j              column-weighted copy
PE reduces over partitions with matmuls into three [1,512] PSUM accumulators:
    S_r += w_i^T g      S_c += 1^T gw      C += 1^T v(masked)
Host combines in f64:  loss = 0.5*(S_r + S_c) / C.

All t/p/w values are bf16-quantized identically on host for row scalars and
column data so pairwise terms stay exactly symmetric and w is consistent.
"""

import sys

if "/opt/trn_rl_repo" not in sys.path:
    sys.path.insert(0, "/opt/trn_rl_repo")

import numpy as np
import ml_dtypes

N = 8192
P = 128
N_CORES = 8
NBLOCKS_TOTAL = N // P                 # 64 row blocks globally
SLOTS = 8                              # row blocks per core
LC = N + 3 * P                         # 8576 local (rotated) columns
W_MAX = 33 * P                         # 4224
# per-slot window start / length in the local column layout
SLOT_START = [P * i for i in range(4)] + [N // 2 + P * i for i in range(4)]
SLOT_LEN = [33 * P] * 4 + [32 * P] * 4
# load-balance knobs (block slots)
RELU_ON_DVE = set()
PD_ON_ACT = set()

_CACHE = {}


def _core_blocks(core):
    return [4 * core + i for i in range(4)] + [32 + 4 * core + i for i in range(4)]


def _window(I):
    n = 33 if I <= 31 else 32
    return [(I + j) % NBLOCKS_TOTAL for j in range(n)]


def _mm_chunks(start, end):
    """Yield (f0, f1) pieces of [start, end) with width <= 512."""
    f = start
    while f < end:
        yield f, min(f + 512, end)
        f = min(f + 512, end)


def _build():
    from contextlib import ExitStack
    from concourse import bacc, tile, mybir

    BF16 = mybir.dt.bfloat16
    F32 = mybir.dt.float32
    Alu = mybir.AluOpType
    Act = mybir.ActivationFunctionType

    nc = bacc.Bacc("TRN2", target_bir_lowering=False, debug=False,
                   num_devices=N_CORES)

    tql_ext = nc.dram_tensor("tql", [1, LC], BF16, kind="ExternalInput").ap()
    pql_ext = nc.dram_tensor("pql", [1, LC], BF16, kind="ExternalInput").ap()
    ti_ext = nc.dram_tensor("ti", [P, SLOTS], F32, kind="ExternalInput").ap()
    nti_ext = nc.dram_tensor("nti", [P, SLOTS], F32, kind="ExternalInput").ap()
    pi_ext = nc.dram_tensor("pi", [P, SLOTS], F32, kind="ExternalInput").ap()
    npi_ext = nc.dram_tensor("npi", [P, SLOTS], F32, kind="ExternalInput").ap()
    wib_ext = nc.dram_tensor("wib", [P, SLOTS], BF16, kind="ExternalInput").ap()
    um_ext = nc.dram_tensor("um", [P, P], BF16, kind="ExternalInput").ap()
    wcp_ext = nc.dram_tensor("wcp", [P, 33 * SLOTS], F32,
                             kind="ExternalInput").ap()
    # cst columns: 0=theta, 1=lo, 2=hi
    cst_ext = nc.dram_tensor("cst", [P, 4], F32, kind="ExternalInput").ap()
    out_ext = nc.dram_tensor("out", [1, 1536], F32, kind="ExternalOutput").ap()

    with tile.TileContext(nc) as tc:
        with ExitStack() as ctx:
            singles = ctx.enter_context(tc.tile_pool(name="singles", bufs=1))
            work = ctx.enter_context(tc.tile_pool(name="work", bufs=2))
            small = ctx.enter_context(tc.tile_pool(name="small", bufs=4))
            psum = ctx.enter_context(tc.tile_pool(name="psum", bufs=1, space="PSUM"))

            ti_sb = singles.tile([P, SLOTS], F32)
            nc.sync.dma_start(out=ti_sb[:], in_=ti_ext[:])
            nti_sb = singles.tile([P, SLOTS], F32)
            nc.sync.dma_start(out=nti_sb[:], in_=nti_ext[:])
            pi_sb = singles.tile([P, SLOTS], F32)
            nc.sync.dma_start(out=pi_sb[:], in_=pi_ext[:])
            npi_sb = singles.tile([P, SLOTS], F32)
            nc.sync.dma_start(out=npi_sb[:], in_=npi_ext[:])
            wib_sb = singles.tile([P, SLOTS], BF16)
            nc.sync.dma_start(out=wib_sb[:], in_=wib_ext[:])
            um_sb = singles.tile([P, P], BF16)
            nc.sync.dma_start(out=um_sb[:], in_=um_ext[:])
            wcp_sb = singles.tile([P, 33 * SLOTS], F32)
            nc.sync.dma_start(out=wcp_sb[:], in_=wcp_ext[:])
            cst_sb = singles.tile([P, 4], F32)
            nc.sync.dma_start(out=cst_sb[:], in_=cst_ext[:])
            ones_sb = singles.tile([P, 1], BF16)
            nc.gpsimd.memset(ones_sb[:], 1.0)
            onesf_sb = singles.tile([P, 1], F32)
            nc.gpsimd.memset(onesf_sb[:], 1.0)
            zerob_sb = singles.tile([P, 1], BF16)
            nc.gpsimd.memset(zerob_sb[:], 0.0)
            zerof_sb = singles.tile([P, 1], F32)
            nc.gpsimd.memset(zerof_sb[:], 0.0)

            tqb = singles.tile([P, LC], BF16)
            pqb = singles.tile([P, LC], BF16)
            # broadcast: fine column chunks spread over DMA queues; tqb
            # issued first (first compute dependency), wcb last
            BCH = LC // 8  # 1072
            for dst, src_ in ((tqb, tql_ext), (pqb, pql_ext)):
                for c0 in range(0, LC, BCH):
                    sl = slice(c0, c0 + BCH)
                    nc.sync.dma_start(out=dst[:, sl],
                                      in_=src_[:, sl].to_broadcast([P, BCH]))


            ps_Sr = psum.tile([1, 512], F32)
            ps_Sc = psum.tile([1, 64], F32)
            ps_C = psum.tile([1, 512], F32)

            for b in range(SLOTS):
                st, L = SLOT_START[b], SLOT_LEN[b]
                csl = slice(st, st + L)
                ad = work.tile([P, L], BF16, tag="ad", bufs=2)
                nc.scalar.activation(out=ad[:], in_=tqb[:, csl], func=Act.Abs,
                                     bias=nti_sb[:, b:b + 1], scale=1.0)
                s = work.tile([P, L], BF16, tag="s", bufs=2)
                nc.scalar.activation(out=s[:], in_=tqb[:, csl], func=Act.Sign,
                                     bias=ti_sb[:, b:b + 1], scale=-1.0)
                m = work.tile([P, L], BF16, tag="m", bufs=1)
                nc.vector.tensor_scalar(
                    out=m[:], in0=ad[:],
                    scalar1=cst_sb[:, 1:2], scalar2=cst_sb[:, 2:3],
                    op0=Alu.max, op1=Alu.min)
                v = work.tile([P, L], BF16, tag="v", bufs=2)
                nc.vector.tensor_scalar(
                    out=v[:], in0=ad[:], scalar1=cst_sb[:, 0:1], scalar2=None,
                    op0=Alu.is_ge)
                pd = work.tile([P, L], BF16, tag="pd", bufs=1)
                if b in PD_ON_ACT:
                    nc.scalar.activation(out=pd[:], in_=pqb[:, csl],
                                         func=Act.Identity,
                                         bias=npi_sb[:, b:b + 1], scale=1.0)
                else:
                    nc.vector.tensor_scalar(
                        out=pd[:], in0=pqb[:, csl], scalar1=pi_sb[:, b:b + 1],
                        scalar2=None, op0=Alu.subtract)
                q = work.tile([P, L], BF16, tag="q", bufs=1)
                nc.vector.tensor_tensor(out=q[:], in0=pd[:], in1=s[:],
                                        op=Alu.mult)
                vp = work.tile([P, L], BF16, tag="vp", bufs=1)
                nc.vector.tensor_tensor(out=vp[:], in0=q[:], in1=m[:],
                                        op=Alu.add)
                # strict-upper mask for the leading diagonal block
                vm = small.tile([P, P], BF16, tag="vm")
                nc.vector.tensor_tensor(out=vm[:], in0=v[:, 0:P], in1=um_sb[:],
                                        op=Alu.mult)
                g = work.tile([P, L], BF16, tag="g", bufs=2)
                if b in RELU_ON_DVE:
                    h = work.tile([P, L], BF16, tag="h", bufs=1)
                    nc.vector.tensor_tensor(out=h[:, 0:P], in0=vp[:, 0:P],
                                            in1=vm[:], op=Alu.mult)
                    nc.vector.tensor_tensor(out=h[:, P:L], in0=vp[:, P:L],
                                            in1=v[:, P:L], op=Alu.mult)
                    nc.vector.tensor_scalar(
                        out=g[:], in0=h[:], scalar1=0.0, scalar2=None,
                        op0=Alu.max)
                else:
                    viol = work.tile([P, L], BF16, tag="viol", bufs=2)
                    nc.scalar.activation(out=viol[:], in_=vp[:], func=Act.Relu)
                    nc.vector.tensor_tensor(out=g[:, 0:P], in0=viol[:, 0:P],
                                            in1=vm[:], op=Alu.mult)
                    nc.vector.tensor_tensor(out=g[:, P:L], in0=viol[:, P:L],
                                            in1=v[:, P:L], op=Alu.mult)

                # PE partition reductions into PSUM accumulators.  start=True
                # only on each accumulator's first full-width matmul.
                for ci, (f0, f1) in enumerate(_mm_chunks(0, L)):
                    nc.tensor.matmul(ps_Sr[:, 0:f1 - f0],
                                     lhsT=wib_sb[:, b:b + 1],
                                     rhs=g[:, f0:f1],
                                     start=(b == 0 and ci == 0),
                                     stop=(b == SLOTS - 1 and f1 == L))
                # column-weighted sum: transpose-reduce g into per-column
                # sums (g-chunk as stationary, ones moving), then dot with
                # the on-partition column weights
                nchunk = L // P
                ps_col = psum.tile([P, 33], F32, tag="ps_col", bufs=2)
                for c in range(nchunk):
                    nc.tensor.matmul(ps_col[:, c:c + 1],
                                     lhsT=g[:, c * P:(c + 1) * P],
                                     rhs=ones_sb[:],
                                     start=True, stop=True)
                # idempotent re-writes of the first two columns: push the
                # PE->PSUM writeback of the tail columns ahead of the read
                for c in (0, 1):
                    nc.tensor.matmul(ps_col[:, c:c + 1],
                                     lhsT=g[:, c * P:(c + 1) * P],
                                     rhs=ones_sb[:],
                                     start=True, stop=True)
                colT = small.tile([P, 33], F32, tag="colT", bufs=2)
                nc.vector.tensor_copy(out=colT[:, 0:nchunk],
                                      in_=ps_col[:, 0:nchunk])
                prod = small.tile([P, 33], F32, tag="prod", bufs=2)
                nc.vector.tensor_tensor(
                    out=prod[:, 0:nchunk], in0=colT[:, 0:nchunk],
                    in1=wcp_sb[:, 33 * b:33 * b + nchunk], op=Alu.mult)
                nc.tensor.matmul(ps_Sc[:, 0:nchunk], lhsT=onesf_sb[:],
                                 rhs=prod[:, 0:nchunk],
                                 start=(b == 0), stop=(b == SLOTS - 1))
                for ci, (f0, f1) in enumerate(_mm_chunks(P, L)):
                    nc.tensor.matmul(ps_C[:, 0:f1 - f0], lhsT=ones_sb[:],
                                     rhs=v[:, f0:f1],
                                     start=(b == 0 and ci == 0), stop=False)
                nc.tensor.matmul(ps_C[:, 0:P], lhsT=ones_sb[:], rhs=vm[:],
                                 start=False, stop=(b == SLOTS - 1))

            # zero-lhsT drain-pusher matmuls: accumulate +0 into each
            # PSUM accumulator so prior writebacks land before the copies
            nc.tensor.matmul(ps_Sr[:, 0:512], lhsT=zerob_sb[:],
                             rhs=g[:, 0:512], start=False, stop=True,
                             skip_group_check=True)
            nc.tensor.matmul(ps_Sc[:, 0:33], lhsT=zerof_sb[:],
                             rhs=prod[:, 0:33], start=False, stop=True,
                             skip_group_check=True)
            nc.tensor.matmul(ps_C[:, 0:512], lhsT=zerob_sb[:],
                             rhs=v[:, P:P + 512], start=False, stop=True,
                             skip_group_check=True)

            out_sb = singles.tile([1, 1536], F32)
            nc.scalar.copy(out=out_sb[0:1, 0:512], in_=ps_Sr[:])
            nc.scalar.copy(out=out_sb[0:1, 512:576], in_=ps_Sc[:])
            nc.scalar.copy(out=out_sb[0:1, 1024:1536], in_=ps_C[:])
            nc.sync.dma_start(out=out_ext[:], in_=out_sb[:])

    nc.compile()
    return nc


def _get_nc():
    if "nc" not in _CACHE:
        _CACHE["nc"] = _build()
    return _CACHE["nc"]


def _prepare_in_maps(predictions, targets, snr_weights, margin_scale):
    ms = float(margin_scale)
    bf16 = ml_dtypes.bfloat16

    t = np.asarray(targets, np.float32)
    p = np.asarray(predictions, np.float32)
    w = np.asarray(snr_weights, np.float32)

    # bf16-quantize once; identical values feed column data and row scalars so
    # every pairwise term is exactly symmetric.
    tq = (0.08 * ms * t).astype(bf16)
    pq = p.astype(bf16)
    wq = w.astype(bf16)
    tqf = tq.astype(np.float32)
    pqf = pq.astype(np.float32)

    cst = np.zeros((P, 4), np.float32)
    cst[:, 0] = np.float32(0.05 * 0.08 * ms)
    cst[:, 1] = np.float32(0.1 * 0.08 * ms)
    cst[:, 2] = np.float32(1.0 * 0.08 * ms)

    um = np.triu(np.ones((P, P), np.float32), k=1).astype(bf16)

    in_maps = []
    for core in range(N_CORES):
        rot = 4 * core * P
        # rotated layout + 3-block tail so every window is contiguous
        idx = (rot + np.arange(LC)) % N
        tql = tq[idx].reshape(1, LC)
        pql = pq[idx].reshape(1, LC)
        blocks = _core_blocks(core)
        wcp = np.zeros((P, 33 * SLOTS), np.float32)
        wqf = wq.astype(np.float32)
        for slot, I in enumerate(blocks):
            win = _window(I)
            for c, J in enumerate(win):
                wcp[:, 33 * slot + c] = wqf[J * P:(J + 1) * P]
        ti = np.empty((P, SLOTS), np.float32)
        pi = np.empty((P, SLOTS), np.float32)
        wib = np.empty((P, SLOTS), np.float32)
        for slot, I in enumerate(blocks):
            rows = slice(I * P, (I + 1) * P)
            ti[:, slot] = tqf[rows]
            pi[:, slot] = pqf[rows]
            wib[:, slot] = wq[rows]
        in_maps.append({
            "tql": tql, "pql": pql,
            "ti": ti, "nti": -ti, "pi": pi, "npi": -pi,
            "wib": wib.astype(bf16),
            "um": um, "cst": cst, "wcp": wcp,
        })
    return in_maps


def kernel(predictions, targets, snr_weights, margin_scale):
    from concourse.bass_utils import run_bass_kernel_spmd

    nc = _get_nc()
    in_maps = _prepare_in_maps(predictions, targets, snr_weights, margin_scale)
    res = run_bass_kernel_spmd(nc, in_maps, core_ids=list(range(N_CORES)))

    Sr = 0.0
    Sc = 0.0
    C = 0.0
    for r in res.results:
        o = np.asarray(r["out"][0], np.float64)
        Sr += float(o[0:512].sum())
        Sc += float(o[512:576].sum())
        C += float(o[1024:1536].sum())
    loss = 0.5 * (Sr + Sc) / C if C > 0 else 0.0
    return np.float32(loss)


# revision 16
# speedup vs baseline: 1.0590x; 1.0590x over previous
"""AdaptiveRankingLoss on 8 Trainium2 NeuronCores (Bass/Tile), upper-triangle v3.

Math
----
reference:  loss = sum_{i<j, |t_i-t_j|>=0.05} 0.5*(w_i+w_j)*relu(-sign(td)*pd + m) / count
            td = t_i - t_j, pd = p_i - p_j, m = ms*0.08*clip(|td|, 0.1, 1.0)

Every per-pair factor is symmetric in i<->j, so each unordered pair is computed
once.  The 64x64 grid of 128-row blocks is covered by a circulant schedule:
row-block I processes column-blocks J in the wrapped window [I, I+n_I) mod 64,
n_I = 33 for I<=31 and 32 for I>=32; every unordered block pair lands in
exactly one window (pair {I,J}, d=J-I: d<=32 -> I's window, else J's), and the
diagonal block leads each window (strict-upper mask there).  Core k owns blocks
{4k..4k+3} and {32+4k..32+4k+3}: identical shapes and work on every core.

Column data is laid out per-core ROTATED by 4k blocks, with the first 3 blocks
duplicated as a tail, so every window is one contiguous slice of a single
[128, 8576] broadcast tile per tensor (slot i<=3: start 128*i len 4224;
slot i>=4: start 4096+128*(i-4) len 4096).

Per block (rows on partitions, window cols on free), all tensors bf16:
    ACT: ad  = Abs( tq_j - tq_i )     tq = bf16(0.08*ms*t)
    ACT: s   = Sign( tq_i - tq_j )
    DVE: m   = (ad max lo) min hi     margin (lo=0.008ms, hi=0.08ms)
    DVE: v   = (ad is_ge theta)      theta = 0.004ms  (<=> |td| >= 0.05)
    DVE: pd  = pq_j - pq_i           [movable to ACT per-block]
    DVE: q   = pd * s
    DVE: vp  = q + m
    ACT: viol= Relu(vp)              [movable to DVE per-block]
    DVE: vm  = v[:,0:128] * U        strict-upper diag mask
    DVE: g   = viol * v              (vm on the leading 128 cols)
    DVE: gw  = g * wc_j              column-weighted copy
PE reduces over partitions with matmuls into three [1,512] PSUM accumulators:
    S_r += w_i^T g      S_c += 1^T gw      C += 1^T v(masked)
Host combines in f64:  loss = 0.5*(S_r + S_c) / C.

All t/p/w values are bf16-quantized identically on host for row scalars and
column data so pairwise terms stay exactly symmetric and w is consistent.
"""

import sys

if "/opt/trn_rl_repo" not in sys.path:
    sys.path.insert(0, "/opt/trn_rl_repo")

import numpy as np
import ml_dtypes

N = 8192
P = 128
N_CORES = 8
NBLOCKS_TOTAL = N // P                 # 64 row blocks globally
SLOTS = 8                              # row blocks per core
LC = N + 3 * P                         # 8576 local (rotated) columns
W_MAX = 33 * P                         # 4224
# per-slot window start / length in the local column layout
SLOT_START = [P * i for i in range(4)] + [N // 2 + P * i for i in range(4)]
SLOT_LEN = [33 * P] * 4 + [32 * P] * 4
# load-balance knobs (block slots)
RELU_ON_DVE = set()
PD_ON_ACT = set()

_CACHE = {}


def _core_blocks(core):
    return [4 * core + i for i in range(4)] + [32 + 4 * core + i for i in range(4)]


def _window(I):
    n = 33 if I <= 31 else 32
    return [(I + j) % NBLOCKS_TOTAL for j in range(n)]


def _mm_chunks(start, end):
    """Yield (f0, f1) pieces of [start, end) with width <= 512."""
    f = start
    while f < end:
        yield f, min(f + 512, end)
        f = min(f + 512, end)


def _build():
    from contextlib import ExitStack
    from concourse import bacc, tile, mybir

    BF16 = mybir.dt.bfloat16
    F32 = mybir.dt.float32
    Alu = mybir.AluOpType
    Act = mybir.ActivationFunctionType

    nc = bacc.Bacc("TRN2", target_bir_lowering=False, debug=False,
                   num_devices=N_CORES)

    tql_ext = nc.dram_tensor("tql", [1, LC], BF16, kind="ExternalInput").ap()
    pql_ext = nc.dram_tensor("pql", [1, LC], BF16, kind="ExternalInput").ap()
    ti_ext = nc.dram_tensor("ti", [P, SLOTS], F32, kind="ExternalInput").ap()
    nti_ext = nc.dram_tensor("nti", [P, SLOTS], F32, kind="ExternalInput").ap()
    pi_ext = nc.dram_tensor("pi", [P, SLOTS], F32, kind="ExternalInput").ap()
    npi_ext = nc.dram_tensor("npi", [P, SLOTS], F32, kind="ExternalInput").ap()
    wib_ext = nc.dram_tensor("wib", [P, SLOTS], BF16, kind="ExternalInput").ap()
    um_ext = nc.dram_tensor("um", [P, P], BF16, kind="ExternalInput").ap()
    wcp_ext = nc.dram_tensor("wcp", [P, 33 * SLOTS], F32,
                             kind="ExternalInput").ap()
    # cst columns: 0=theta, 1=lo, 2=hi
    cst_ext = nc.dram_tensor("cst", [P, 4], F32, kind="ExternalInput").ap()
    out_ext = nc.dram_tensor("out", [1, 1536], F32, kind="ExternalOutput").ap()

    with tile.TileContext(nc) as tc:
        with ExitStack() as ctx:
            singles = ctx.enter_context(tc.tile_pool(name="singles", bufs=1))
            work = ctx.enter_context(tc.tile_pool(name="work", bufs=2))
            small = ctx.enter_context(tc.tile_pool(name="small", bufs=4))
            psum = ctx.enter_context(tc.tile_pool(name="psum", bufs=1, space="PSUM"))

            ti_sb = singles.tile([P, SLOTS], F32)
            nc.sync.dma_start(out=ti_sb[:], in_=ti_ext[:])
            nti_sb = singles.tile([P, SLOTS], F32)
            nc.sync.dma_start(out=nti_sb[:], in_=nti_ext[:])
            pi_sb = singles.tile([P, SLOTS], F32)
            nc.sync.dma_start(out=pi_sb[:], in_=pi_ext[:])
            npi_sb = singles.tile([P, SLOTS], F32)
            nc.sync.dma_start(out=npi_sb[:], in_=npi_ext[:])
            wib_sb = singles.tile([P, SLOTS], BF16)
            nc.sync.dma_start(out=wib_sb[:], in_=wib_ext[:])
            um_sb = singles.tile([P, P], BF16)
            nc.sync.dma_start(out=um_sb[:], in_=um_ext[:])
            wcp_sb = singles.tile([P, 33 * SLOTS], F32)
            nc.sync.dma_start(out=wcp_sb[:], in_=wcp_ext[:])
            cst_sb = singles.tile([P, 4], F32)
            nc.sync.dma_start(out=cst_sb[:], in_=cst_ext[:])
            ones_sb = singles.tile([P, 1], BF16)
            nc.gpsimd.memset(ones_sb[:], 1.0)
            onesf_sb = singles.tile([P, 1], F32)
            nc.gpsimd.memset(onesf_sb[:], 1.0)
            zerob_sb = singles.tile([P, 1], BF16)
            nc.gpsimd.memset(zerob_sb[:], 0.0)
            zerof_sb = singles.tile([P, 1], F32)
            nc.gpsimd.memset(zerof_sb[:], 0.0)

            tqb = singles.tile([P, LC], BF16)
            pqb = singles.tile([P, LC], BF16)
            # broadcast: column chunks x 4-way partition split so each DMA
            # carries only 32 descriptors (descriptor-rate limited); tqb
            # chunks issued first (first compute dependency)
            BCH = LC // 8  # 1072
            for dst, src_ in ((tqb, tql_ext), (pqb, pql_ext)):
                for c0 in range(0, LC, BCH):
                    sl = slice(c0, c0 + BCH)
                    for p0 in range(0, P, 32):
                        nc.sync.dma_start(
                            out=dst[p0:p0 + 32, sl],
                            in_=src_[:, sl].to_broadcast([32, BCH]))


            ps_Sr = psum.tile([1, 512], F32)
            ps_Sc = psum.tile([1, 64], F32)
            ps_C = psum.tile([1, 512], F32)

            for b in range(SLOTS):
                st, L = SLOT_START[b], SLOT_LEN[b]
                csl = slice(st, st + L)
                ad = work.tile([P, L], BF16, tag="ad", bufs=2)
                nc.scalar.activation(out=ad[:], in_=tqb[:, csl], func=Act.Abs,
                                     bias=nti_sb[:, b:b + 1], scale=1.0)
                s = work.tile([P, L], BF16, tag="s", bufs=2)
                nc.scalar.activation(out=s[:], in_=tqb[:, csl], func=Act.Sign,
                                     bias=ti_sb[:, b:b + 1], scale=-1.0)
                m = work.tile([P, L], BF16, tag="m", bufs=1)
                nc.vector.tensor_scalar(
                    out=m[:], in0=ad[:],
                    scalar1=cst_sb[:, 1:2], scalar2=cst_sb[:, 2:3],
                    op0=Alu.max, op1=Alu.min)
                v = work.tile([P, L], BF16, tag="v", bufs=2)
                nc.vector.tensor_scalar(
                    out=v[:], in0=ad[:], scalar1=cst_sb[:, 0:1], scalar2=None,
                    op0=Alu.is_ge)
                pd = work.tile([P, L], BF16, tag="pd", bufs=1)
                if b in PD_ON_ACT:
                    nc.scalar.activation(out=pd[:], in_=pqb[:, csl],
                                         func=Act.Identity,
                                         bias=npi_sb[:, b:b + 1], scale=1.0)
                else:
                    nc.vector.tensor_scalar(
                        out=pd[:], in0=pqb[:, csl], scalar1=pi_sb[:, b:b + 1],
                        scalar2=None, op0=Alu.subtract)
                q = work.tile([P, L], BF16, tag="q", bufs=1)
                nc.vector.tensor_tensor(out=q[:], in0=pd[:], in1=s[:],
                                        op=Alu.mult)
                vp = work.tile([P, L], BF16, tag="vp", bufs=1)
                nc.vector.tensor_tensor(out=vp[:], in0=q[:], in1=m[:],
                                        op=Alu.add)
                # strict-upper mask for the leading diagonal block
                vm = small.tile([P, P], BF16, tag="vm")
                nc.vector.tensor_tensor(out=vm[:], in0=v[:, 0:P], in1=um_sb[:],
                                        op=Alu.mult)
                g = work.tile([P, L], BF16, tag="g", bufs=2)
                if b in RELU_ON_DVE:
                    h = work.tile([P, L], BF16, tag="h", bufs=1)
                    nc.vector.tensor_tensor(out=h[:, 0:P], in0=vp[:, 0:P],
                                            in1=vm[:], op=Alu.mult)
                    nc.vector.tensor_tensor(out=h[:, P:L], in0=vp[:, P:L],
                                            in1=v[:, P:L], op=Alu.mult)
                    nc.vector.tensor_scalar(
                        out=g[:], in0=h[:], scalar1=0.0, scalar2=None,
                        op0=Alu.max)
                else:
                    viol = work.tile([P, L], BF16, tag="viol", bufs=2)
                    nc.scalar.activation(out=viol[:], in_=vp[:], func=Act.Relu)
                    nc.vector.tensor_tensor(out=g[:, 0:P], in0=viol[:, 0:P],
                                            in1=vm[:], op=Alu.mult)
                    nc.vector.tensor_tensor(out=g[:, P:L], in0=viol[:, P:L],
                                            in1=v[:, P:L], op=Alu.mult)

                # PE partition reductions into PSUM accumulators.  start=True
                # only on each accumulator's first full-width matmul.
                for ci, (f0, f1) in enumerate(_mm_chunks(0, L)):
                    nc.tensor.matmul(ps_Sr[:, 0:f1 - f0],
                                     lhsT=wib_sb[:, b:b + 1],
                                     rhs=g[:, f0:f1],
                                     start=(b == 0 and ci == 0),
                                     stop=(b == SLOTS - 1 and f1 == L))
                # column-weighted sum: transpose-reduce g into per-column
                # sums (g-chunk as stationary, ones moving), then dot with
                # the on-partition column weights
                nchunk = L // P
                ps_col = psum.tile([P, 33], F32, tag="ps_col", bufs=2)
                for c in range(nchunk):
                    nc.tensor.matmul(ps_col[:, c:c + 1],
                                     lhsT=g[:, c * P:(c + 1) * P],
                                     rhs=ones_sb[:],
                                     start=True, stop=True)
                # idempotent re-writes of the first two columns: push the
                # PE->PSUM writeback of the tail columns ahead of the read
                for c in (0, 1):
                    nc.tensor.matmul(ps_col[:, c:c + 1],
                                     lhsT=g[:, c * P:(c + 1) * P],
                                     rhs=ones_sb[:],
                                     start=True, stop=True)
                colT = small.tile([P, 33], F32, tag="colT", bufs=2)
                nc.vector.tensor_copy(out=colT[:, 0:nchunk],
                                      in_=ps_col[:, 0:nchunk])
                prod = small.tile([P, 33], F32, tag="prod", bufs=2)
                nc.vector.tensor_tensor(
                    out=prod[:, 0:nchunk], in0=colT[:, 0:nchunk],
                    in1=wcp_sb[:, 33 * b:33 * b + nchunk], op=Alu.mult)
                nc.tensor.matmul(ps_Sc[:, 0:nchunk], lhsT=onesf_sb[:],
                                 rhs=prod[:, 0:nchunk],
                                 start=(b == 0), stop=(b == SLOTS - 1))
                for ci, (f0, f1) in enumerate(_mm_chunks(P, L)):
                    nc.tensor.matmul(ps_C[:, 0:f1 - f0], lhsT=ones_sb[:],
                                     rhs=v[:, f0:f1],
                                     start=(b == 0 and ci == 0), stop=False)
                nc.tensor.matmul(ps_C[:, 0:P], lhsT=ones_sb[:], rhs=vm[:],
                                 start=False, stop=(b == SLOTS - 1))

            # zero-lhsT drain-pusher matmuls: accumulate +0 into each
            # PSUM accumulator so prior writebacks land before the copies
            nc.tensor.matmul(ps_Sr[:, 0:512], lhsT=zerob_sb[:],
                             rhs=g[:, 0:512], start=False, stop=True,
                             skip_group_check=True)
            nc.tensor.matmul(ps_Sc[:, 0:33], lhsT=zerof_sb[:],
                             rhs=prod[:, 0:33], start=False, stop=True,
                             skip_group_check=True)
            nc.tensor.matmul(ps_C[:, 0:512], lhsT=zerob_sb[:],
                             rhs=v[:, P:P + 512], start=False, stop=True,
                             skip_group_check=True)

            out_sb = singles.tile([1, 1536], F32)
            nc.scalar.copy(out=out_sb[0:1, 0:512], in_=ps_Sr[:])
            nc.scalar.copy(out=out_sb[0:1, 512:576], in_=ps_Sc[:])
            nc.scalar.copy(out=out_sb[0:1, 1024:1536], in_=ps_C[:])
            nc.sync.dma_start(out=out_ext[:], in_=out_sb[:])

    nc.compile()
    return nc


def _get_nc():
    if "nc" not in _CACHE:
        _CACHE["nc"] = _build()
    return _CACHE["nc"]


def _prepare_in_maps(predictions, targets, snr_weights, margin_scale):
    ms = float(margin_scale)
    bf16 = ml_dtypes.bfloat16

    t = np.asarray(targets, np.float32)
    p = np.asarray(predictions, np.float32)
    w = np.asarray(snr_weights, np.float32)

    # bf16-quantize once; identical values feed column data and row scalars so
    # every pairwise term is exactly symmetric.
    tq = (0.08 * ms * t).astype(bf16)
    pq = p.astype(bf16)
    wq = w.astype(bf16)
    tqf = tq.astype(np.float32)
    pqf = pq.astype(np.float32)

    cst = np.zeros((P, 4), np.float32)
    cst[:, 0] = np.float32(0.05 * 0.08 * ms)
    cst[:, 1] = np.float32(0.1 * 0.08 * ms)
    cst[:, 2] = np.float32(1.0 * 0.08 * ms)

    um = np.triu(np.ones((P, P), np.float32), k=1).astype(bf16)

    in_maps = []
    for core in range(N_CORES):
        rot = 4 * core * P
        # rotated layout + 3-block tail so every window is contiguous
        idx = (rot + np.arange(LC)) % N
        tql = tq[idx].reshape(1, LC)
        pql = pq[idx].reshape(1, LC)
        blocks = _core_blocks(core)
        wcp = np.zeros((P, 33 * SLOTS), np.float32)
        wqf = wq.astype(np.float32)
        for slot, I in enumerate(blocks):
            win = _window(I)
            for c, J in enumerate(win):
                wcp[:, 33 * slot + c] = wqf[J * P:(J + 1) * P]
        ti = np.empty((P, SLOTS), np.float32)
        pi = np.empty((P, SLOTS), np.float32)
        wib = np.empty((P, SLOTS), np.float32)
        for slot, I in enumerate(blocks):
            rows = slice(I * P, (I + 1) * P)
            ti[:, slot] = tqf[rows]
            pi[:, slot] = pqf[rows]
            wib[:, slot] = wq[rows]
        in_maps.append({
            "tql": tql, "pql": pql,
            "ti": ti, "nti": -ti, "pi": pi, "npi": -pi,
            "wib": wib.astype(bf16),
            "um": um, "cst": cst, "wcp": wcp,
        })
    return in_maps


def kernel(predictions, targets, snr_weights, margin_scale):
    from concourse.bass_utils import run_bass_kernel_spmd

    nc = _get_nc()
    in_maps = _prepare_in_maps(predictions, targets, snr_weights, margin_scale)
    res = run_bass_kernel_spmd(nc, in_maps, core_ids=list(range(N_CORES)))

    Sr = 0.0
    Sc = 0.0
    C = 0.0
    for r in res.results:
        o = np.asarray(r["out"][0], np.float64)
        Sr += float(o[0:512].sum())
        Sc += float(o[512:576].sum())
        C += float(o[1024:1536].sum())
    loss = 0.5 * (Sr + Sc) / C if C > 0 else 0.0
    return np.float32(loss)


# revision 17
# speedup vs baseline: 1.1759x; 1.1104x over previous
"""AdaptiveRankingLoss on 8 Trainium2 NeuronCores (Bass/Tile), upper-triangle v3.

Math
----
reference:  loss = sum_{i<j, |t_i-t_j|>=0.05} 0.5*(w_i+w_j)*relu(-sign(td)*pd + m) / count
            td = t_i - t_j, pd = p_i - p_j, m = ms*0.08*clip(|td|, 0.1, 1.0)

Every per-pair factor is symmetric in i<->j, so each unordered pair is computed
once.  The 64x64 grid of 128-row blocks is covered by a circulant schedule:
row-block I processes column-blocks J in the wrapped window [I, I+n_I) mod 64,
n_I = 33 for I<=31 and 32 for I>=32; every unordered block pair lands in
exactly one window (pair {I,J}, d=J-I: d<=32 -> I's window, else J's), and the
diagonal block leads each window (strict-upper mask there).  Core k owns blocks
{4k..4k+3} and {32+4k..32+4k+3}: identical shapes and work on every core.

Column data is laid out per-core ROTATED by 4k blocks, with the first 3 blocks
duplicated as a tail, so every window is one contiguous slice of a single
[128, 8576] broadcast tile per tensor (slot i<=3: start 128*i len 4224;
slot i>=4: start 4096+128*(i-4) len 4096).

Per block (rows on partitions, window cols on free), all tensors bf16:
    ACT: ad  = Abs( tq_j - tq_i )     tq = bf16(0.08*ms*t)
    ACT: s   = Sign( tq_i - tq_j )
    DVE: m   = (ad max lo) min hi     margin (lo=0.008ms, hi=0.08ms)
    DVE: v   = (ad is_ge theta)      theta = 0.004ms  (<=> |td| >= 0.05)
    DVE: pd  = pq_j - pq_i           [movable to ACT per-block]
    DVE: q   = pd * s
    DVE: vp  = q + m
    ACT: viol= Relu(vp)              [movable to DVE per-block]
    DVE: vm  = v[:,0:128] * U        strict-upper diag mask
    DVE: g   = viol * v              (vm on the leading 128 cols)
    DVE: gw  = g * wc_j              column-weighted copy
PE reduces over partitions with matmuls into three [1,512] PSUM accumulators:
    S_r += w_i^T g      S_c += 1^T gw      C += 1^T v(masked)
Host combines in f64:  loss = 0.5*(S_r + S_c) / C.

All t/p/w values are bf16-quantized identically on host for row scalars and
column data so pairwise terms stay exactly symmetric and w is consistent.
"""

import sys

if "/opt/trn_rl_repo" not in sys.path:
    sys.path.insert(0, "/opt/trn_rl_repo")

import numpy as np
import ml_dtypes

N = 8192
P = 128
N_CORES = 8
NBLOCKS_TOTAL = N // P                 # 64 row blocks globally
SLOTS = 8                              # row blocks per core
LC = N + 3 * P                         # 8576 local (rotated) columns
W_MAX = 33 * P                         # 4224
# per-slot window start / length in the local column layout
SLOT_START = [P * i for i in range(4)] + [N // 2 + P * i for i in range(4)]
SLOT_LEN = [33 * P] * 4 + [32 * P] * 4
# load-balance knobs (block slots)
RELU_ON_DVE = set()
PD_ON_ACT = set()

_CACHE = {}


def _core_blocks(core):
    return [4 * core + i for i in range(4)] + [32 + 4 * core + i for i in range(4)]


def _window(I):
    n = 33 if I <= 31 else 32
    return [(I + j) % NBLOCKS_TOTAL for j in range(n)]


def _mm_chunks(start, end):
    """Yield (f0, f1) pieces of [start, end) with width <= 512."""
    f = start
    while f < end:
        yield f, min(f + 512, end)
        f = min(f + 512, end)


def _build():
    from contextlib import ExitStack
    from concourse import bacc, tile, mybir

    BF16 = mybir.dt.bfloat16
    F32 = mybir.dt.float32
    Alu = mybir.AluOpType
    Act = mybir.ActivationFunctionType

    nc = bacc.Bacc("TRN2", target_bir_lowering=False, debug=False,
                   num_devices=N_CORES)

    tql_ext = nc.dram_tensor("tql", [P, LC], BF16, kind="ExternalInput").ap()
    pql_ext = nc.dram_tensor("pql", [P, LC], BF16, kind="ExternalInput").ap()
    ti_ext = nc.dram_tensor("ti", [P, SLOTS], F32, kind="ExternalInput").ap()
    nti_ext = nc.dram_tensor("nti", [P, SLOTS], F32, kind="ExternalInput").ap()
    pi_ext = nc.dram_tensor("pi", [P, SLOTS], F32, kind="ExternalInput").ap()
    npi_ext = nc.dram_tensor("npi", [P, SLOTS], F32, kind="ExternalInput").ap()
    wib_ext = nc.dram_tensor("wib", [P, SLOTS], BF16, kind="ExternalInput").ap()
    um_ext = nc.dram_tensor("um", [P, P], BF16, kind="ExternalInput").ap()
    wcp_ext = nc.dram_tensor("wcp", [P, 33 * SLOTS], F32,
                             kind="ExternalInput").ap()
    # cst columns: 0=theta, 1=lo, 2=hi
    cst_ext = nc.dram_tensor("cst", [P, 4], F32, kind="ExternalInput").ap()
    out_ext = nc.dram_tensor("out", [1, 1536], F32, kind="ExternalOutput").ap()

    with tile.TileContext(nc) as tc:
        with ExitStack() as ctx:
            singles = ctx.enter_context(tc.tile_pool(name="singles", bufs=1))
            work = ctx.enter_context(tc.tile_pool(name="work", bufs=2))
            small = ctx.enter_context(tc.tile_pool(name="small", bufs=4))
            psum = ctx.enter_context(tc.tile_pool(name="psum", bufs=1, space="PSUM"))

            ti_sb = singles.tile([P, SLOTS], F32)
            nc.sync.dma_start(out=ti_sb[:], in_=ti_ext[:])
            nti_sb = singles.tile([P, SLOTS], F32)
            nc.sync.dma_start(out=nti_sb[:], in_=nti_ext[:])
            pi_sb = singles.tile([P, SLOTS], F32)
            nc.sync.dma_start(out=pi_sb[:], in_=pi_ext[:])
            npi_sb = singles.tile([P, SLOTS], F32)
            nc.sync.dma_start(out=npi_sb[:], in_=npi_ext[:])
            wib_sb = singles.tile([P, SLOTS], BF16)
            nc.sync.dma_start(out=wib_sb[:], in_=wib_ext[:])
            um_sb = singles.tile([P, P], BF16)
            nc.sync.dma_start(out=um_sb[:], in_=um_ext[:])
            wcp_sb = singles.tile([P, 33 * SLOTS], F32)
            nc.sync.dma_start(out=wcp_sb[:], in_=wcp_ext[:])
            cst_sb = singles.tile([P, 4], F32)
            nc.sync.dma_start(out=cst_sb[:], in_=cst_ext[:])
            ones_sb = singles.tile([P, 1], BF16)
            nc.gpsimd.memset(ones_sb[:], 1.0)
            onesf_sb = singles.tile([P, 1], F32)
            nc.gpsimd.memset(onesf_sb[:], 1.0)
            zerob_sb = singles.tile([P, 1], BF16)
            nc.gpsimd.memset(zerob_sb[:], 0.0)
            zerof_sb = singles.tile([P, 1], F32)
            nc.gpsimd.memset(zerof_sb[:], 0.0)

            tqb = singles.tile([P, LC], BF16)
            pqb = singles.tile([P, LC], BF16)
            # column data arrives host-pre-broadcast: plain contiguous
            # DMAs (HWDGE-friendly), chunked for early compute start; tqb
            # first (first compute dependency)
            BCH = LC // 4  # 2144
            for dst, src_ in ((tqb, tql_ext), (pqb, pql_ext)):
                for c0 in range(0, LC, BCH):
                    sl = slice(c0, c0 + BCH)
                    nc.sync.dma_start(out=dst[:, sl], in_=src_[:, sl])


            ps_Sr = psum.tile([1, 512], F32)
            ps_Sc = psum.tile([1, 64], F32)
            ps_C = psum.tile([1, 512], F32)

            for b in range(SLOTS):
                st, L = SLOT_START[b], SLOT_LEN[b]
                csl = slice(st, st + L)
                ad = work.tile([P, L], BF16, tag="ad", bufs=2)
                nc.scalar.activation(out=ad[:], in_=tqb[:, csl], func=Act.Abs,
                                     bias=nti_sb[:, b:b + 1], scale=1.0)
                s = work.tile([P, L], BF16, tag="s", bufs=2)
                nc.scalar.activation(out=s[:], in_=tqb[:, csl], func=Act.Sign,
                                     bias=ti_sb[:, b:b + 1], scale=-1.0)
                m = work.tile([P, L], BF16, tag="m", bufs=1)
                nc.vector.tensor_scalar(
                    out=m[:], in0=ad[:],
                    scalar1=cst_sb[:, 1:2], scalar2=cst_sb[:, 2:3],
                    op0=Alu.max, op1=Alu.min)
                v = work.tile([P, L], BF16, tag="v", bufs=2)
                nc.vector.tensor_scalar(
                    out=v[:], in0=ad[:], scalar1=cst_sb[:, 0:1], scalar2=None,
                    op0=Alu.is_ge)
                pd = work.tile([P, L], BF16, tag="pd", bufs=1)
                if b in PD_ON_ACT:
                    nc.scalar.activation(out=pd[:], in_=pqb[:, csl],
                                         func=Act.Identity,
                                         bias=npi_sb[:, b:b + 1], scale=1.0)
                else:
                    nc.vector.tensor_scalar(
                        out=pd[:], in0=pqb[:, csl], scalar1=pi_sb[:, b:b + 1],
                        scalar2=None, op0=Alu.subtract)
                q = work.tile([P, L], BF16, tag="q", bufs=1)
                nc.vector.tensor_tensor(out=q[:], in0=pd[:], in1=s[:],
                                        op=Alu.mult)
                vp = work.tile([P, L], BF16, tag="vp", bufs=1)
                nc.vector.tensor_tensor(out=vp[:], in0=q[:], in1=m[:],
                                        op=Alu.add)
                # strict-upper mask for the leading diagonal block
                vm = small.tile([P, P], BF16, tag="vm")
                nc.vector.tensor_tensor(out=vm[:], in0=v[:, 0:P], in1=um_sb[:],
                                        op=Alu.mult)
                g = work.tile([P, L], BF16, tag="g", bufs=2)
                if b in RELU_ON_DVE:
                    h = work.tile([P, L], BF16, tag="h", bufs=1)
                    nc.vector.tensor_tensor(out=h[:, 0:P], in0=vp[:, 0:P],
                                            in1=vm[:], op=Alu.mult)
                    nc.vector.tensor_tensor(out=h[:, P:L], in0=vp[:, P:L],
                                            in1=v[:, P:L], op=Alu.mult)
                    nc.vector.tensor_scalar(
                        out=g[:], in0=h[:], scalar1=0.0, scalar2=None,
                        op0=Alu.max)
                else:
                    viol = work.tile([P, L], BF16, tag="viol", bufs=2)
                    nc.scalar.activation(out=viol[:], in_=vp[:], func=Act.Relu)
                    nc.vector.tensor_tensor(out=g[:, 0:P], in0=viol[:, 0:P],
                                            in1=vm[:], op=Alu.mult)
                    nc.vector.tensor_tensor(out=g[:, P:L], in0=viol[:, P:L],
                                            in1=v[:, P:L], op=Alu.mult)

                # PE partition reductions into PSUM accumulators.  start=True
                # only on each accumulator's first full-width matmul.
                for ci, (f0, f1) in enumerate(_mm_chunks(0, L)):
                    nc.tensor.matmul(ps_Sr[:, 0:f1 - f0],
                                     lhsT=wib_sb[:, b:b + 1],
                                     rhs=g[:, f0:f1],
                                     start=(b == 0 and ci == 0),
                                     stop=(b == SLOTS - 1 and f1 == L))
                # column-weighted sum: transpose-reduce g into per-column
                # sums (g-chunk as stationary, ones moving), then dot with
                # the on-partition column weights
                nchunk = L // P
                ps_col = psum.tile([P, 33], F32, tag="ps_col", bufs=2)
                for c in range(nchunk):
                    nc.tensor.matmul(ps_col[:, c:c + 1],
                                     lhsT=g[:, c * P:(c + 1) * P],
                                     rhs=ones_sb[:],
                                     start=True, stop=True)
                # idempotent re-writes of the first two columns: push the
                # PE->PSUM writeback of the tail columns ahead of the read
                for c in (0, 1):
                    nc.tensor.matmul(ps_col[:, c:c + 1],
                                     lhsT=g[:, c * P:(c + 1) * P],
                                     rhs=ones_sb[:],
                                     start=True, stop=True)
                colT = small.tile([P, 33], F32, tag="colT", bufs=2)
                nc.vector.tensor_copy(out=colT[:, 0:nchunk],
                                      in_=ps_col[:, 0:nchunk])
                prod = small.tile([P, 33], F32, tag="prod", bufs=2)
                nc.vector.tensor_tensor(
                    out=prod[:, 0:nchunk], in0=colT[:, 0:nchunk],
                    in1=wcp_sb[:, 33 * b:33 * b + nchunk], op=Alu.mult)
                nc.tensor.matmul(ps_Sc[:, 0:nchunk], lhsT=onesf_sb[:],
                                 rhs=prod[:, 0:nchunk],
                                 start=(b == 0), stop=(b == SLOTS - 1))
                for ci, (f0, f1) in enumerate(_mm_chunks(P, L)):
                    nc.tensor.matmul(ps_C[:, 0:f1 - f0], lhsT=ones_sb[:],
                                     rhs=v[:, f0:f1],
                                     start=(b == 0 and ci == 0), stop=False)
                nc.tensor.matmul(ps_C[:, 0:P], lhsT=ones_sb[:], rhs=vm[:],
                                 start=False, stop=(b == SLOTS - 1))

            # zero-lhsT drain-pusher matmuls: accumulate +0 into each
            # PSUM accumulator so prior writebacks land before the copies
            nc.tensor.matmul(ps_Sr[:, 0:512], lhsT=zerob_sb[:],
                             rhs=g[:, 0:512], start=False, stop=True,
                             skip_group_check=True)
            nc.tensor.matmul(ps_Sc[:, 0:33], lhsT=zerof_sb[:],
                             rhs=prod[:, 0:33], start=False, stop=True,
                             skip_group_check=True)
            nc.tensor.matmul(ps_C[:, 0:512], lhsT=zerob_sb[:],
                             rhs=v[:, P:P + 512], start=False, stop=True,
                             skip_group_check=True)

            out_sb = singles.tile([1, 1536], F32)
            nc.scalar.copy(out=out_sb[0:1, 0:512], in_=ps_Sr[:])
            nc.scalar.copy(out=out_sb[0:1, 512:576], in_=ps_Sc[:])
            nc.scalar.copy(out=out_sb[0:1, 1024:1536], in_=ps_C[:])
            nc.sync.dma_start(out=out_ext[:], in_=out_sb[:])

    nc.compile()
    return nc


def _get_nc():
    if "nc" not in _CACHE:
        _CACHE["nc"] = _build()
    return _CACHE["nc"]


def _prepare_in_maps(predictions, targets, snr_weights, margin_scale):
    ms = float(margin_scale)
    bf16 = ml_dtypes.bfloat16

    t = np.asarray(targets, np.float32)
    p = np.asarray(predictions, np.float32)
    w = np.asarray(snr_weights, np.float32)

    # bf16-quantize once; identical values feed column data and row scalars so
    # every pairwise term is exactly symmetric.
    tq = (0.08 * ms * t).astype(bf16)
    pq = p.astype(bf16)
    wq = w.astype(bf16)
    tqf = tq.astype(np.float32)
    pqf = pq.astype(np.float32)

    cst = np.zeros((P, 4), np.float32)
    cst[:, 0] = np.float32(0.05 * 0.08 * ms)
    cst[:, 1] = np.float32(0.1 * 0.08 * ms)
    cst[:, 2] = np.float32(1.0 * 0.08 * ms)

    um = np.triu(np.ones((P, P), np.float32), k=1).astype(bf16)

    in_maps = []
    for core in range(N_CORES):
        rot = 4 * core * P
        # rotated layout + 3-block tail so every window is contiguous
        idx = (rot + np.arange(LC)) % N
        tql = np.ascontiguousarray(
            np.broadcast_to(tq[idx].reshape(1, LC), (P, LC)))
        pql = np.ascontiguousarray(
            np.broadcast_to(pq[idx].reshape(1, LC), (P, LC)))
        blocks = _core_blocks(core)
        wcp = np.zeros((P, 33 * SLOTS), np.float32)
        wqf = wq.astype(np.float32)
        for slot, I in enumerate(blocks):
            win = _window(I)
            for c, J in enumerate(win):
                wcp[:, 33 * slot + c] = wqf[J * P:(J + 1) * P]
        ti = np.empty((P, SLOTS), np.float32)
        pi = np.empty((P, SLOTS), np.float32)
        wib = np.empty((P, SLOTS), np.float32)
        for slot, I in enumerate(blocks):
            rows = slice(I * P, (I + 1) * P)
            ti[:, slot] = tqf[rows]
            pi[:, slot] = pqf[rows]
            wib[:, slot] = wq[rows]
        in_maps.append({
            "tql": tql, "pql": pql,
            "ti": ti, "nti": -ti, "pi": pi, "npi": -pi,
            "wib": wib.astype(bf16),
            "um": um, "cst": cst, "wcp": wcp,
        })
    return in_maps


def kernel(predictions, targets, snr_weights, margin_scale):
    from concourse.bass_utils import run_bass_kernel_spmd

    nc = _get_nc()
    in_maps = _prepare_in_maps(predictions, targets, snr_weights, margin_scale)
    res = run_bass_kernel_spmd(nc, in_maps, core_ids=list(range(N_CORES)))

    Sr = 0.0
    Sc = 0.0
    C = 0.0
    for r in res.results:
        o = np.asarray(r["out"][0], np.float64)
        Sr += float(o[0:512].sum())
        Sc += float(o[512:576].sum())
        C += float(o[1024:1536].sum())
    loss = 0.5 * (Sr + Sc) / C if C > 0 else 0.0
    return np.float32(loss)


# revision 21
# speedup vs baseline: 1.2616x; 1.0728x over previous
"""AdaptiveRankingLoss on 8 Trainium2 NeuronCores (Bass/Tile), upper-triangle v3.

Math
----
reference:  loss = sum_{i<j, |t_i-t_j|>=0.05} 0.5*(w_i+w_j)*relu(-sign(td)*pd + m) / count
            td = t_i - t_j, pd = p_i - p_j, m = ms*0.08*clip(|td|, 0.1, 1.0)

Every per-pair factor is symmetric in i<->j, so each unordered pair is computed
once.  The 64x64 grid of 128-row blocks is covered by a circulant schedule:
row-block I processes column-blocks J in the wrapped window [I, I+n_I) mod 64,
n_I = 33 for I<=31 and 32 for I>=32; every unordered block pair lands in
exactly one window (pair {I,J}, d=J-I: d<=32 -> I's window, else J's), and the
diagonal block leads each window (strict-upper mask there).  Core k owns blocks
{4k..4k+3} and {32+4k..32+4k+3}: identical shapes and work on every core.

Column data is laid out per-core ROTATED by 4k blocks, with the first 3 blocks
duplicated as a tail, so every window is one contiguous slice of a single
[128, 8576] broadcast tile per tensor (slot i<=3: start 128*i len 4224;
slot i>=4: start 4096+128*(i-4) len 4096).

Per block (rows on partitions, window cols on free), all tensors bf16:
    ACT: ad  = Abs( tq_j - tq_i )     tq = bf16(0.08*ms*t)
    ACT: s   = Sign( tq_i - tq_j )
    DVE: m   = (ad max lo) min hi     margin (lo=0.008ms, hi=0.08ms)
    DVE: v   = (ad is_ge theta)      theta = 0.004ms  (<=> |td| >= 0.05)
    DVE: pd  = pq_j - pq_i           [movable to ACT per-block]
    DVE: q   = pd * s
    DVE: vp  = q + m
    ACT: viol= Relu(vp)              [movable to DVE per-block]
    DVE: vm  = v[:,0:128] * U        strict-upper diag mask
    DVE: g   = viol * v              (vm on the leading 128 cols)
    DVE: gw  = g * wc_j              column-weighted copy
PE reduces over partitions with matmuls into three [1,512] PSUM accumulators:
    S_r += w_i^T g      S_c += 1^T gw      C += 1^T v(masked)
Host combines in f64:  loss = 0.5*(S_r + S_c) / C.

All t/p/w values are bf16-quantized identically on host for row scalars and
column data so pairwise terms stay exactly symmetric and w is consistent.
"""

import sys

if "/opt/trn_rl_repo" not in sys.path:
    sys.path.insert(0, "/opt/trn_rl_repo")

import numpy as np
import ml_dtypes

N = 8192
P = 128
N_CORES = 8
NBLOCKS_TOTAL = N // P                 # 64 row blocks globally
SLOTS = 8                              # row blocks per core
LC = N + 3 * P                         # 8576 local (rotated) columns
W_MAX = 33 * P                         # 4224
# per-slot window start / length in the local column layout
SLOT_START = [P * i for i in range(4)] + [N // 2 + P * i for i in range(4)]
SLOT_LEN = [33 * P] * 4 + [32 * P] * 4
# load-balance knobs (block slots)
RELU_ON_DVE = set()
PD_ON_ACT = {3, 6}

_CACHE = {}


def _core_blocks(core):
    return [4 * core + i for i in range(4)] + [32 + 4 * core + i for i in range(4)]


def _window(I):
    n = 33 if I <= 31 else 32
    return [(I + j) % NBLOCKS_TOTAL for j in range(n)]


def _mm_chunks(start, end):
    """Yield (f0, f1) pieces of [start, end) with width <= 512."""
    f = start
    while f < end:
        yield f, min(f + 512, end)
        f = min(f + 512, end)


def _build():
    from contextlib import ExitStack
    from concourse import bacc, tile, mybir

    BF16 = mybir.dt.bfloat16
    F32 = mybir.dt.float32
    Alu = mybir.AluOpType
    Act = mybir.ActivationFunctionType

    nc = bacc.Bacc("TRN2", target_bir_lowering=False, debug=False,
                   num_devices=N_CORES)

    tql_ext = nc.dram_tensor("tql", [P, LC], BF16, kind="ExternalInput").ap()
    pql_ext = nc.dram_tensor("pql", [P, LC], BF16, kind="ExternalInput").ap()
    # aux f32: 0:8 ti | 8:16 nti | 16:24 pi | 24:32 npi | 32:36 cst | 36:300 wcp
    aux_ext = nc.dram_tensor("aux", [P, 300], F32, kind="ExternalInput").ap()
    # auxb bf16: 0:8 wib | 8:136 um
    auxb_ext = nc.dram_tensor("auxb", [P, 136], BF16, kind="ExternalInput").ap()
    out_ext = nc.dram_tensor("out", [1, 1536], F32, kind="ExternalOutput").ap()

    with tile.TileContext(nc) as tc:
        with ExitStack() as ctx:
            singles = ctx.enter_context(tc.tile_pool(name="singles", bufs=1))
            work = ctx.enter_context(tc.tile_pool(name="work", bufs=2))
            small = ctx.enter_context(tc.tile_pool(name="small", bufs=4))
            psum = ctx.enter_context(tc.tile_pool(name="psum", bufs=1, space="PSUM"))

            aux_sb = singles.tile([P, 300], F32)
            nc.scalar.dma_start(out=aux_sb[:], in_=aux_ext[:])
            auxb_sb = singles.tile([P, 136], BF16)
            nc.scalar.dma_start(out=auxb_sb[:], in_=auxb_ext[:])

            ones_sb = singles.tile([P, 1], BF16)
            nc.gpsimd.memset(ones_sb[:], 1.0)
            onesf_sb = singles.tile([P, 1], F32)
            nc.gpsimd.memset(onesf_sb[:], 1.0)
            zerob_sb = singles.tile([P, 1], BF16)
            nc.gpsimd.memset(zerob_sb[:], 0.0)
            zerof_sb = singles.tile([P, 1], F32)
            nc.gpsimd.memset(zerof_sb[:], 0.0)

            tqb = singles.tile([P, LC], BF16)
            pqb = singles.tile([P, LC], BF16)
            # column data arrives host-pre-broadcast: plain contiguous
            # DMAs (HWDGE-friendly), chunked for early compute start; tqb
            # first (first compute dependency)
            BCH = LC // 4  # 2144
            for eng, dst, src_ in ((nc.sync, tqb, tql_ext),
                                   (nc.gpsimd, pqb, pql_ext)):
                for c0 in range(0, LC, BCH):
                    sl = slice(c0, c0 + BCH)
                    eng.dma_start(out=dst[:, sl], in_=src_[:, sl])


            ps_Sr = psum.tile([1, 512], F32)
            ps_Sc = psum.tile([1, 64], F32)
            ps_C = psum.tile([1, 512], F32)

            for b in range(SLOTS):
                st, L = SLOT_START[b], SLOT_LEN[b]
                csl = slice(st, st + L)
                ad = work.tile([P, L], BF16, tag="ad", bufs=3)
                nc.scalar.activation(out=ad[:], in_=tqb[:, csl], func=Act.Abs,
                                     bias=aux_sb[:, 8 + b:9 + b], scale=1.0)
                s = work.tile([P, L], BF16, tag="s", bufs=3)
                nc.scalar.activation(out=s[:], in_=tqb[:, csl], func=Act.Sign,
                                     bias=aux_sb[:, b:b + 1], scale=-1.0)
                m = work.tile([P, L], BF16, tag="m", bufs=1)
                nc.vector.tensor_scalar(
                    out=m[:], in0=ad[:],
                    scalar1=aux_sb[:, 33:34], scalar2=aux_sb[:, 34:35],
                    op0=Alu.max, op1=Alu.min)
                v = work.tile([P, L], BF16, tag="v", bufs=2)
                nc.vector.tensor_scalar(
                    out=v[:], in0=ad[:], scalar1=aux_sb[:, 32:33], scalar2=None,
                    op0=Alu.is_ge)
                pd = work.tile([P, L], BF16, tag="pd", bufs=2)
                if b in PD_ON_ACT:
                    nc.scalar.activation(out=pd[:], in_=pqb[:, csl],
                                         func=Act.Identity,
                                         bias=aux_sb[:, 24 + b:25 + b], scale=1.0)
                else:
                    nc.vector.tensor_scalar(
                        out=pd[:], in0=pqb[:, csl], scalar1=aux_sb[:, 16 + b:17 + b],
                        scalar2=None, op0=Alu.subtract)
                q = work.tile([P, L], BF16, tag="q", bufs=1)
                nc.vector.tensor_tensor(out=q[:], in0=pd[:], in1=s[:],
                                        op=Alu.mult)
                vp = work.tile([P, L], BF16, tag="vp", bufs=2)
                nc.vector.tensor_tensor(out=vp[:], in0=q[:], in1=m[:],
                                        op=Alu.add)
                # strict-upper mask for the leading diagonal block
                vm = small.tile([P, P], BF16, tag="vm")
                nc.vector.tensor_tensor(out=vm[:], in0=v[:, 0:P], in1=auxb_sb[:, 8:136],
                                        op=Alu.mult)
                g = work.tile([P, L], BF16, tag="g", bufs=2)
                if b in RELU_ON_DVE:
                    h = work.tile([P, L], BF16, tag="h", bufs=1)
                    nc.vector.tensor_tensor(out=h[:, 0:P], in0=vp[:, 0:P],
                                            in1=vm[:], op=Alu.mult)
                    nc.vector.tensor_tensor(out=h[:, P:L], in0=vp[:, P:L],
                                            in1=v[:, P:L], op=Alu.mult)
                    nc.vector.tensor_scalar(
                        out=g[:], in0=h[:], scalar1=0.0, scalar2=None,
                        op0=Alu.max)
                else:
                    viol = work.tile([P, L], BF16, tag="viol", bufs=2)
                    nc.scalar.activation(out=viol[:], in_=vp[:], func=Act.Relu)
                    nc.vector.tensor_tensor(out=g[:, 0:P], in0=viol[:, 0:P],
                                            in1=vm[:], op=Alu.mult)
                    nc.vector.tensor_tensor(out=g[:, P:L], in0=viol[:, P:L],
                                            in1=v[:, P:L], op=Alu.mult)

                # PE partition reductions into PSUM accumulators.  start=True
                # only on each accumulator's first full-width matmul.
                for ci, (f0, f1) in enumerate(_mm_chunks(0, L)):
                    nc.tensor.matmul(ps_Sr[:, 0:f1 - f0],
                                     lhsT=auxb_sb[:, b:b + 1],
                                     rhs=g[:, f0:f1],
                                     start=(b == 0 and ci == 0),
                                     stop=(b == SLOTS - 1 and f1 == L))
                # column-weighted sum: transpose-reduce g into per-column
                # sums (g-chunk as stationary, ones moving), then dot with
                # the on-partition column weights
                nchunk = L // P
                ps_col = psum.tile([P, 33], F32, tag="ps_col", bufs=2)
                col_mms = []
                for c in range(nchunk):
                    col_mms.append(nc.tensor.matmul(
                        ps_col[:, c:c + 1],
                        lhsT=g[:, c * P:(c + 1) * P],
                        rhs=ones_sb[:],
                        start=True, stop=True))
                # idempotent re-writes of the first two columns: push the
                # PE->PSUM writeback of the tail columns ahead of the read
                for c in (0, 1):
                    col_mms.append(nc.tensor.matmul(
                        ps_col[:, c:c + 1],
                        lhsT=g[:, c * P:(c + 1) * P],
                        rhs=ones_sb[:],
                        start=True, stop=True))
                colT = small.tile([P, 33], F32, tag="colT", bufs=2)
                cp = nc.vector.tensor_copy(out=colT[:, 0:nchunk],
                                           in_=ps_col[:, 0:nchunk])
                for mm_i in col_mms:
                    tile.add_dep_helper(
                        cp.ins, mm_i.ins,
                        reason="colT copy waits all ps_col matmuls")
                prod = small.tile([P, 33], F32, tag="prod", bufs=2)
                nc.vector.tensor_tensor(
                    out=prod[:, 0:nchunk], in0=colT[:, 0:nchunk],
                    in1=aux_sb[:, 36 + 33 * b:36 + 33 * b + nchunk], op=Alu.mult)
                nc.tensor.matmul(ps_Sc[:, 0:nchunk], lhsT=onesf_sb[:],
                                 rhs=prod[:, 0:nchunk],
                                 start=(b == 0), stop=(b == SLOTS - 1))
                for ci, (f0, f1) in enumerate(_mm_chunks(P, L)):
                    nc.tensor.matmul(ps_C[:, 0:f1 - f0], lhsT=ones_sb[:],
                                     rhs=v[:, f0:f1],
                                     start=(b == 0 and ci == 0), stop=False)
                nc.tensor.matmul(ps_C[:, 0:P], lhsT=ones_sb[:], rhs=vm[:],
                                 start=False, stop=(b == SLOTS - 1))

            pu1 = nc.tensor.matmul(ps_Sr[:, 0:512], lhsT=zerob_sb[:],
                                   rhs=g[:, 0:512], start=False, stop=True,
                                   skip_group_check=True)
            pu2 = nc.tensor.matmul(ps_Sc[:, 0:33], lhsT=zerof_sb[:],
                                   rhs=prod[:, 0:33], start=False, stop=True,
                                   skip_group_check=True)
            pu3 = nc.tensor.matmul(ps_C[:, 0:512], lhsT=zerob_sb[:],
                                   rhs=v[:, P:P + 512], start=False, stop=True,
                                   skip_group_check=True)
            out_sb = singles.tile([1, 1536], F32)
            c1 = nc.scalar.copy(out=out_sb[0:1, 0:512], in_=ps_Sr[:])
            c2 = nc.scalar.copy(out=out_sb[0:1, 512:576], in_=ps_Sc[:])
            c3 = nc.scalar.copy(out=out_sb[0:1, 1024:1536], in_=ps_C[:])
            for cc, pp in ((c1, pu1), (c2, pu2), (c3, pu3)):
                tile.add_dep_helper(cc.ins, pp.ins,
                                    reason="final copy waits drain pusher")
            nc.sync.dma_start(out=out_ext[:], in_=out_sb[:])

    nc.compile()
    return nc


def _get_nc():
    if "nc" not in _CACHE:
        _CACHE["nc"] = _build()
    return _CACHE["nc"]


def _prepare_in_maps(predictions, targets, snr_weights, margin_scale):
    ms = float(margin_scale)
    bf16 = ml_dtypes.bfloat16

    t = np.asarray(targets, np.float32)
    p = np.asarray(predictions, np.float32)
    w = np.asarray(snr_weights, np.float32)

    # bf16-quantize once; identical values feed column data and row scalars so
    # every pairwise term is exactly symmetric.
    tq = (0.08 * ms * t).astype(bf16)
    pq = p.astype(bf16)
    wq = w.astype(bf16)
    tqf = tq.astype(np.float32)
    pqf = pq.astype(np.float32)

    cst = np.zeros((P, 4), np.float32)
    cst[:, 0] = np.float32(0.05 * 0.08 * ms)
    cst[:, 1] = np.float32(0.1 * 0.08 * ms)
    cst[:, 2] = np.float32(1.0 * 0.08 * ms)

    um = np.triu(np.ones((P, P), np.float32), k=1).astype(bf16)

    in_maps = []
    for core in range(N_CORES):
        rot = 4 * core * P
        # rotated layout + 3-block tail so every window is contiguous
        idx = (rot + np.arange(LC)) % N
        tql = np.ascontiguousarray(
            np.broadcast_to(tq[idx].reshape(1, LC), (P, LC)))
        pql = np.ascontiguousarray(
            np.broadcast_to(pq[idx].reshape(1, LC), (P, LC)))
        blocks = _core_blocks(core)
        wcp = np.zeros((P, 33 * SLOTS), np.float32)
        wqf = wq.astype(np.float32)
        ti = np.empty((P, SLOTS), np.float32)
        pi = np.empty((P, SLOTS), np.float32)
        wib = np.empty((P, SLOTS), np.float32)
        for slot, I in enumerate(blocks):
            win = _window(I)
            for c, J in enumerate(win):
                wcp[:, 33 * slot + c] = wqf[J * P:(J + 1) * P]
            rows = slice(I * P, (I + 1) * P)
            ti[:, slot] = tqf[rows]
            pi[:, slot] = pqf[rows]
            wib[:, slot] = wq[rows]
        aux = np.concatenate([ti, -ti, pi, -pi, cst, wcp], axis=1)
        auxb = np.concatenate([wib.astype(bf16).astype(np.float32), um.astype(np.float32)],
                              axis=1).astype(bf16)
        in_maps.append({"tql": tql, "pql": pql, "aux": aux.astype(np.float32),
                        "auxb": auxb})
    return in_maps


def kernel(predictions, targets, snr_weights, margin_scale):
    from concourse.bass_utils import run_bass_kernel_spmd

    nc = _get_nc()
    in_maps = _prepare_in_maps(predictions, targets, snr_weights, margin_scale)
    res = run_bass_kernel_spmd(nc, in_maps, core_ids=list(range(N_CORES)))

    Sr = 0.0
    Sc = 0.0
    C = 0.0
    for r in res.results:
        o = np.asarray(r["out"][0], np.float64)
        Sr += float(o[0:512].sum())
        Sc += float(o[512:576].sum())
        C += float(o[1024:1536].sum())
    loss = 0.5 * (Sr + Sc) / C if C > 0 else 0.0
    return np.float32(loss)


# revision 22
# speedup vs baseline: 1.2905x; 1.0229x over previous
"""AdaptiveRankingLoss on 8 Trainium2 NeuronCores (Bass/Tile), upper-triangle v3.

Math
----
reference:  loss = sum_{i<j, |t_i-t_j|>=0.05} 0.5*(w_i+w_j)*relu(-sign(td)*pd + m) / count
            td = t_i - t_j, pd = p_i - p_j, m = ms*0.08*clip(|td|, 0.1, 1.0)

Every per-pair factor is symmetric in i<->j, so each unordered pair is computed
once.  The 64x64 grid of 128-row blocks is covered by a circulant schedule:
row-block I processes column-blocks J in the wrapped window [I, I+n_I) mod 64,
n_I = 33 for I<=31 and 32 for I>=32; every unordered block pair lands in
exactly one window (pair {I,J}, d=J-I: d<=32 -> I's window, else J's), and the
diagonal block leads each window (strict-upper mask there).  Core k owns blocks
{4k..4k+3} and {32+4k..32+4k+3}: identical shapes and work on every core.

Column data is laid out per-core ROTATED by 4k blocks, with the first 3 blocks
duplicated as a tail, so every window is one contiguous slice of a single
[128, 8576] broadcast tile per tensor (slot i<=3: start 128*i len 4224;
slot i>=4: start 4096+128*(i-4) len 4096).

Per block (rows on partitions, window cols on free), all tensors bf16:
    ACT: ad  = Abs( tq_j - tq_i )     tq = bf16(0.08*ms*t)
    ACT: s   = Sign( tq_i - tq_j )
    DVE: m   = (ad max lo) min hi     margin (lo=0.008ms, hi=0.08ms)
    DVE: v   = (ad is_ge theta)      theta = 0.004ms  (<=> |td| >= 0.05)
    DVE: pd  = pq_j - pq_i           [movable to ACT per-block]
    DVE: q   = pd * s
    DVE: vp  = q + m
    ACT: viol= Relu(vp)              [movable to DVE per-block]
    DVE: vm  = v[:,0:128] * U        strict-upper diag mask
    DVE: g   = viol * v              (vm on the leading 128 cols)
    DVE: gw  = g * wc_j              column-weighted copy
PE reduces over partitions with matmuls into three [1,512] PSUM accumulators:
    S_r += w_i^T g      S_c += 1^T gw      C += 1^T v(masked)
Host combines in f64:  loss = 0.5*(S_r + S_c) / C.

All t/p/w values are bf16-quantized identically on host for row scalars and
column data so pairwise terms stay exactly symmetric and w is consistent.
"""

import sys

if "/opt/trn_rl_repo" not in sys.path:
    sys.path.insert(0, "/opt/trn_rl_repo")

import numpy as np
import ml_dtypes

N = 8192
P = 128
N_CORES = 8
NBLOCKS_TOTAL = N // P                 # 64 row blocks globally
SLOTS = 8                              # row blocks per core
LC = N + 3 * P                         # 8576 local (rotated) columns
W_MAX = 33 * P                         # 4224
# per-slot window start / length in the local column layout
SLOT_START = [P * i for i in range(4)] + [N // 2 + P * i for i in range(4)]
SLOT_LEN = [33 * P] * 4 + [32 * P] * 4
# load-balance knobs (block slots)
RELU_ON_DVE = set()
PD_ON_ACT = set()

_CACHE = {}


def _core_blocks(core):
    return [4 * core + i for i in range(4)] + [32 + 4 * core + i for i in range(4)]


def _window(I):
    n = 33 if I <= 31 else 32
    return [(I + j) % NBLOCKS_TOTAL for j in range(n)]


def _mm_chunks(start, end):
    """Yield (f0, f1) pieces of [start, end) with width <= 512."""
    f = start
    while f < end:
        yield f, min(f + 512, end)
        f = min(f + 512, end)


def _build():
    from contextlib import ExitStack
    from concourse import bacc, tile, mybir

    BF16 = mybir.dt.bfloat16
    F32 = mybir.dt.float32
    Alu = mybir.AluOpType
    Act = mybir.ActivationFunctionType

    nc = bacc.Bacc("TRN2", target_bir_lowering=False, debug=False,
                   num_devices=N_CORES)

    tql_ext = nc.dram_tensor("tql", [P, LC], BF16, kind="ExternalInput").ap()
    pql_ext = nc.dram_tensor("pql", [P, LC], BF16, kind="ExternalInput").ap()
    # aux f32: 0:8 ti | 8:16 nti | 16:24 pi | 24:32 npi | 32:36 cst | 36:300 wcp
    aux_ext = nc.dram_tensor("aux", [P, 300], F32, kind="ExternalInput").ap()
    # auxb bf16: 0:8 wib | 8:136 um
    auxb_ext = nc.dram_tensor("auxb", [P, 136], BF16, kind="ExternalInput").ap()
    out_ext = nc.dram_tensor("out", [1, 1536], F32, kind="ExternalOutput").ap()

    with tile.TileContext(nc) as tc:
        with ExitStack() as ctx:
            singles = ctx.enter_context(tc.tile_pool(name="singles", bufs=1))
            work = ctx.enter_context(tc.tile_pool(name="work", bufs=2))
            small = ctx.enter_context(tc.tile_pool(name="small", bufs=4))
            psum = ctx.enter_context(tc.tile_pool(name="psum", bufs=1, space="PSUM"))

            aux_sb = singles.tile([P, 300], F32)
            nc.scalar.dma_start(out=aux_sb[:], in_=aux_ext[:])
            auxb_sb = singles.tile([P, 136], BF16)
            nc.scalar.dma_start(out=auxb_sb[:], in_=auxb_ext[:])

            ones_sb = singles.tile([P, 1], BF16)
            nc.gpsimd.memset(ones_sb[:], 1.0)
            onesf_sb = singles.tile([P, 1], F32)
            nc.gpsimd.memset(onesf_sb[:], 1.0)
            zerob_sb = singles.tile([P, 1], BF16)
            nc.gpsimd.memset(zerob_sb[:], 0.0)
            zerof_sb = singles.tile([P, 1], F32)
            nc.gpsimd.memset(zerof_sb[:], 0.0)

            tqb = singles.tile([P, LC], BF16)
            pqb = singles.tile([P, LC], BF16)
            # column data arrives host-pre-broadcast: plain contiguous
            # DMAs (HWDGE-friendly), chunked for early compute start; tqb
            # first (first compute dependency)
            BCH = LC // 4  # 2144
            for eng, dst, src_ in ((nc.sync, tqb, tql_ext),
                                   (nc.gpsimd, pqb, pql_ext)):
                for c0 in range(0, LC, BCH):
                    sl = slice(c0, c0 + BCH)
                    eng.dma_start(out=dst[:, sl], in_=src_[:, sl])


            ps_Sr = psum.tile([1, 512], F32)
            ps_Sc = psum.tile([1, 64], F32)
            ps_C = psum.tile([1, 512], F32)

            for b in range(SLOTS):
                st, L = SLOT_START[b], SLOT_LEN[b]
                csl = slice(st, st + L)
                ad = work.tile([P, L], BF16, tag="ad", bufs=3)
                nc.scalar.activation(out=ad[:], in_=tqb[:, csl], func=Act.Abs,
                                     bias=aux_sb[:, 8 + b:9 + b], scale=1.0)
                s = work.tile([P, L], BF16, tag="s", bufs=3)
                nc.scalar.activation(out=s[:], in_=tqb[:, csl], func=Act.Sign,
                                     bias=aux_sb[:, b:b + 1], scale=-1.0)
                m = work.tile([P, L], BF16, tag="m", bufs=1)
                nc.vector.tensor_scalar(
                    out=m[:], in0=ad[:],
                    scalar1=aux_sb[:, 33:34], scalar2=aux_sb[:, 34:35],
                    op0=Alu.max, op1=Alu.min)
                v = work.tile([P, L], BF16, tag="v", bufs=2)
                nc.vector.tensor_scalar(
                    out=v[:], in0=ad[:], scalar1=aux_sb[:, 32:33], scalar2=None,
                    op0=Alu.is_ge)
                pd = work.tile([P, L], BF16, tag="pd", bufs=2)
                if b in PD_ON_ACT:
                    nc.scalar.activation(out=pd[:], in_=pqb[:, csl],
                                         func=Act.Identity,
                                         bias=aux_sb[:, 24 + b:25 + b], scale=1.0)
                else:
                    nc.vector.tensor_scalar(
                        out=pd[:], in0=pqb[:, csl], scalar1=aux_sb[:, 16 + b:17 + b],
                        scalar2=None, op0=Alu.subtract)
                q = work.tile([P, L], BF16, tag="q", bufs=1)
                nc.vector.tensor_tensor(out=q[:], in0=pd[:], in1=s[:],
                                        op=Alu.mult)
                vp = work.tile([P, L], BF16, tag="vp", bufs=2)
                nc.vector.tensor_tensor(out=vp[:], in0=q[:], in1=m[:],
                                        op=Alu.add)
                # strict-upper mask for the leading diagonal block
                vm = small.tile([P, P], BF16, tag="vm")
                nc.vector.tensor_tensor(out=vm[:], in0=v[:, 0:P], in1=auxb_sb[:, 8:136],
                                        op=Alu.mult)
                g = work.tile([P, L], BF16, tag="g", bufs=2)
                if b in RELU_ON_DVE:
                    h = work.tile([P, L], BF16, tag="h", bufs=1)
                    nc.vector.tensor_tensor(out=h[:, 0:P], in0=vp[:, 0:P],
                                            in1=vm[:], op=Alu.mult)
                    nc.vector.tensor_tensor(out=h[:, P:L], in0=vp[:, P:L],
                                            in1=v[:, P:L], op=Alu.mult)
                    nc.vector.tensor_scalar(
                        out=g[:], in0=h[:], scalar1=0.0, scalar2=None,
                        op0=Alu.max)
                else:
                    viol = work.tile([P, L], BF16, tag="viol", bufs=2)
                    nc.scalar.activation(out=viol[:], in_=vp[:], func=Act.Relu)
                    nc.vector.tensor_tensor(out=g[:, 0:P], in0=viol[:, 0:P],
                                            in1=vm[:], op=Alu.mult)
                    nc.vector.tensor_tensor(out=g[:, P:L], in0=viol[:, P:L],
                                            in1=v[:, P:L], op=Alu.mult)

                # PE partition reductions into PSUM accumulators.  start=True
                # only on each accumulator's first full-width matmul.
                for ci, (f0, f1) in enumerate(_mm_chunks(0, L)):
                    nc.tensor.matmul(ps_Sr[:, 0:f1 - f0],
                                     lhsT=auxb_sb[:, b:b + 1],
                                     rhs=g[:, f0:f1],
                                     start=(b == 0 and ci == 0),
                                     stop=(b == SLOTS - 1 and f1 == L))
                # column-weighted sum: transpose-reduce g into per-column
                # sums (g-chunk as stationary, ones moving), then dot with
                # the on-partition column weights
                nchunk = L // P
                ps_col = psum.tile([P, 33], F32, tag="ps_col", bufs=2)
                col_mms = []
                for c in range(nchunk):
                    col_mms.append(nc.tensor.matmul(
                        ps_col[:, c:c + 1],
                        lhsT=g[:, c * P:(c + 1) * P],
                        rhs=ones_sb[:],
                        start=True, stop=True))
                # idempotent re-writes of the first two columns: push the
                # PE->PSUM writeback of the tail columns ahead of the read
                for c in (0, 1):
                    col_mms.append(nc.tensor.matmul(
                        ps_col[:, c:c + 1],
                        lhsT=g[:, c * P:(c + 1) * P],
                        rhs=ones_sb[:],
                        start=True, stop=True))
                colT = small.tile([P, 33], F32, tag="colT", bufs=2)
                cp = nc.vector.tensor_copy(out=colT[:, 0:nchunk],
                                           in_=ps_col[:, 0:nchunk])
                for mm_i in col_mms:
                    tile.add_dep_helper(
                        cp.ins, mm_i.ins,
                        reason="colT copy waits all ps_col matmuls")
                prod = small.tile([P, 33], F32, tag="prod", bufs=2)
                nc.vector.tensor_tensor(
                    out=prod[:, 0:nchunk], in0=colT[:, 0:nchunk],
                    in1=aux_sb[:, 36 + 33 * b:36 + 33 * b + nchunk], op=Alu.mult)
                nc.tensor.matmul(ps_Sc[:, 0:nchunk], lhsT=onesf_sb[:],
                                 rhs=prod[:, 0:nchunk],
                                 start=(b == 0), stop=(b == SLOTS - 1))
                for ci, (f0, f1) in enumerate(_mm_chunks(P, L)):
                    nc.tensor.matmul(ps_C[:, 0:f1 - f0], lhsT=ones_sb[:],
                                     rhs=v[:, f0:f1],
                                     start=(b == 0 and ci == 0), stop=False)
                nc.tensor.matmul(ps_C[:, 0:P], lhsT=ones_sb[:], rhs=vm[:],
                                 start=False, stop=(b == SLOTS - 1))

            pu1 = nc.tensor.matmul(ps_Sr[:, 0:512], lhsT=zerob_sb[:],
                                   rhs=g[:, 0:512], start=False, stop=True,
                                   skip_group_check=True)
            pu2 = nc.tensor.matmul(ps_Sc[:, 0:33], lhsT=zerof_sb[:],
                                   rhs=prod[:, 0:33], start=False, stop=True,
                                   skip_group_check=True)
            pu3 = nc.tensor.matmul(ps_C[:, 0:512], lhsT=zerob_sb[:],
                                   rhs=v[:, P:P + 512], start=False, stop=True,
                                   skip_group_check=True)
            out_sb = singles.tile([1, 1536], F32)
            c1 = nc.scalar.copy(out=out_sb[0:1, 0:512], in_=ps_Sr[:])
            c2 = nc.scalar.copy(out=out_sb[0:1, 512:576], in_=ps_Sc[:])
            c3 = nc.scalar.copy(out=out_sb[0:1, 1024:1536], in_=ps_C[:])
            for cc, pp in ((c1, pu1), (c2, pu2), (c3, pu3)):
                tile.add_dep_helper(cc.ins, pp.ins,
                                    reason="final copy waits drain pusher")
            nc.sync.dma_start(out=out_ext[:], in_=out_sb[:])

    nc.compile()
    return nc


def _get_nc():
    if "nc" not in _CACHE:
        _CACHE["nc"] = _build()
    return _CACHE["nc"]


def _prepare_in_maps(predictions, targets, snr_weights, margin_scale):
    ms = float(margin_scale)
    bf16 = ml_dtypes.bfloat16

    t = np.asarray(targets, np.float32)
    p = np.asarray(predictions, np.float32)
    w = np.asarray(snr_weights, np.float32)

    # bf16-quantize once; identical values feed column data and row scalars so
    # every pairwise term is exactly symmetric.
    tq = (0.08 * ms * t).astype(bf16)
    pq = p.astype(bf16)
    wq = w.astype(bf16)
    tqf = tq.astype(np.float32)
    pqf = pq.astype(np.float32)

    cst = np.zeros((P, 4), np.float32)
    cst[:, 0] = np.float32(0.05 * 0.08 * ms)
    cst[:, 1] = np.float32(0.1 * 0.08 * ms)
    cst[:, 2] = np.float32(1.0 * 0.08 * ms)

    um = np.triu(np.ones((P, P), np.float32), k=1).astype(bf16)

    in_maps = []
    for core in range(N_CORES):
        rot = 4 * core * P
        # rotated layout + 3-block tail so every window is contiguous
        idx = (rot + np.arange(LC)) % N
        tql = np.ascontiguousarray(
            np.broadcast_to(tq[idx].reshape(1, LC), (P, LC)))
        pql = np.ascontiguousarray(
            np.broadcast_to(pq[idx].reshape(1, LC), (P, LC)))
        blocks = _core_blocks(core)
        wcp = np.zeros((P, 33 * SLOTS), np.float32)
        wqf = wq.astype(np.float32)
        ti = np.empty((P, SLOTS), np.float32)
        pi = np.empty((P, SLOTS), np.float32)
        wib = np.empty((P, SLOTS), np.float32)
        for slot, I in enumerate(blocks):
            win = _window(I)
            for c, J in enumerate(win):
                wcp[:, 33 * slot + c] = wqf[J * P:(J + 1) * P]
            rows = slice(I * P, (I + 1) * P)
            ti[:, slot] = tqf[rows]
            pi[:, slot] = pqf[rows]
            wib[:, slot] = wq[rows]
        aux = np.concatenate([ti, -ti, pi, -pi, cst, wcp], axis=1)
        auxb = np.concatenate([wib.astype(bf16).astype(np.float32), um.astype(np.float32)],
                              axis=1).astype(bf16)
        in_maps.append({"tql": tql, "pql": pql, "aux": aux.astype(np.float32),
                        "auxb": auxb})
    return in_maps


def kernel(predictions, targets, snr_weights, margin_scale):
    from concourse.bass_utils import run_bass_kernel_spmd

    nc = _get_nc()
    in_maps = _prepare_in_maps(predictions, targets, snr_weights, margin_scale)
    res = run_bass_kernel_spmd(nc, in_maps, core_ids=list(range(N_CORES)))

    Sr = 0.0
    Sc = 0.0
    C = 0.0
    for r in res.results:
        o = np.asarray(r["out"][0], np.float64)
        Sr += float(o[0:512].sum())
        Sc += float(o[512:576].sum())
        C += float(o[1024:1536].sum())
    loss = 0.5 * (Sr + Sc) / C if C > 0 else 0.0
    return np.float32(loss)
